# revision 46
# baseline (speedup 1.0000x reference)
"""DyanEncoder (FISTA sparse coding) Trainium2 kernel.

Math (3 reweighting rounds x 200 FISTA iterations, all sequential):
    D [36,641],  L = ||D^T D||_2,  E = -D^T/L
    u_i = y_i + E (D y_i - Y) - wl          (rank-36 cascade, c folded in)
    x_{i+1} = relu(u_i) + min(u_i + 2wl, 0) (soft-threshold)
    y_{i+1} = (1+rho_i) x_{i+1} - rho_i x_i (momentum; rho precomputed)

Sharding: data-parallel over P=512 columns, 64 per core, zero comm in the
iteration; one scalar AllReduce per w-renormalization (2 total).

Per-iteration dataflow (per core; K padded 641->768, state [128, 6*64];
momentum recombined in the tiny [36,64] space via wt_k = D@x_k - Y, using
(1+r)wt_new - r wt_old = D@y - Y since (1+r)-r = 1):
    wt = D@x16 - Y                    (6 fp16 MMs + I36@(-Y) -> PSUM [36,64])
    s16 = (1+rho')wt - rho' wt_old    (ln_bwd_dx custom DVE op, fp16 out)
    u(round 1)  = E@s16               (6 fp16 MMs; thresholds are scalar)
    u(rounds2+) = cp*I@x_pair + cm*I@x_old_pair - wl + E@s16
                  (scaled fp16 identities, ScalarE-generated one iteration
                   ahead; x as compensated fp16 hi+lo pair; cp = fp16(1+rho),
                   cm = -(cp-1) exactly so cp+cm == 1)
    x = SHRINK custom DVE op: round 1 relu(u+y+C0)+min(u+y+C1,0) with y via
        Src1 (emitted twice: fp16 out gates the next w~ chain, fp32 after);
        rounds 2+ relu(u)+min(u+w2,0) with tensor w2, then x16/xlo casts.
Numerical ground rules (measured): the y/identity path must keep x in
fp32-equivalent form (plain fp16 there diverges 0.25 rel err via momentum
noise amplification); the residual cascade tolerates fp16 (1.0e-3 final).
Round boundary: wr = 1/(|x|+eps) (fast reciprocal), ssq via Square-ACT
accum + ones-matmul partition reduce, AllReduce, rsqrt, new wl/w2 in fp16.
"""

import math
import os

import numpy as np

import concourse.bass as bass
import concourse.mybir as mybir
import concourse.tile as tile
from concourse import bacc
from concourse.bass_utils import run_bass_kernel_spmd

import concourse.dve_ops as dve_ops
from concourse.dve_spec import Spec, Src0, Src1, Zero, relu, minn, lower, _has_src1
from concourse.dve_uop import DveOpSpec

f32, f16 = mybir.dt.float32, mybir.dt.float16
AF = mybir.ActivationFunctionType

N_CORES = 8
MAX_ITER = int(os.environ.get("FISTA_ITERS", "200"))
N_REWEIGHT = int(os.environ.get("FISTA_ROUNDS", "3"))
EPS_W = 0.01
LAM = 0.1
T_DIM = 36
K_DIM = 641
P_DIM = 512
KP = 768            # padded K (6 blocks of 128)
NB = 6              # K blocks
PL = P_DIM // N_CORES   # 64 local columns
SW = NB * PL        # 384 state width ([128, SW] layout, block j at cols j*PL)


# ---------------- custom fused shrink op ----------------
def _shrink_ref(in0, in1, s0, s1, imm2):
    u = in0.astype(np.float32)
    w2 = in1.astype(np.float32)
    return (np.maximum(u, 0.0) + np.minimum(u + w2, 0.0)).astype(np.float32)


def _register_shrink():
    name = "SHRINK_FISTA_ANT"
    for op in dve_ops.OPS:
        if op.name == name:
            return op
    spec = Spec(body=relu(Src0) + minn(Src0 + Src1, Zero), reference=_shrink_ref)
    row = dve_ops._CUSTOM_DVE_ROW_BASE + len(dve_ops.OPS)
    assert row < 0x20
    dve_ops._SUB_OPCODE_FOR_NAME[name] = row
    shas = {}
    for ver in ("v3", "v4"):
        uops = lower(spec, ver=ver)
        shas[ver] = DveOpSpec(
            name=name, opcode=row, uops=uops, rd1_en=_has_src1(spec)
        ).sha(ver)
    op = dve_ops.DveOp(name, spec, subdim=False, uops_sha=shas)
    dve_ops.OPS.append(op)
    dve_ops.CUSTOM_DVE_SPECS[name] = spec
    return op


def _shrink_r1_ref(in0, in1, s0, s1, imm2):
    u = in0.astype(np.float32) + in1.astype(np.float32)
    return (np.maximum(u + s0, 0.0) + np.minimum(u + s1, 0.0)).astype(np.float32)


def _register_shrink_r1():
    """Round-1 shrink with SCALAR thresholds and the +y folded in:
    out = relu(Src0+Src1+C0) + min(Src0+Src1+C1, 0)
    (in0 = E@s psum, in1 = y32, s0 = -wl, s1 = +wl)."""
    name = "SHRINK_R1_FISTA_ANT"
    for op in dve_ops.OPS:
        if op.name == name:
            return op
    u = Src0 + Src1
    from concourse.dve_spec import C0, C1
    spec = Spec(body=relu(u + C0) + minn(u + C1, Zero), reference=_shrink_r1_ref)
    row = dve_ops._CUSTOM_DVE_ROW_BASE + len(dve_ops.OPS)
    assert row < 0x20
    dve_ops._SUB_OPCODE_FOR_NAME[name] = row
    shas = {}
    for ver in ("v3", "v4"):
        uops = lower(spec, ver=ver)
        shas[ver] = DveOpSpec(
            name=name, opcode=row, uops=uops, rd1_en=_has_src1(spec)
        ).sha(ver)
    op = dve_ops.DveOp(name, spec, subdim=False, uops_sha=shas)
    dve_ops.OPS.append(op)
    dve_ops.CUSTOM_DVE_SPECS[name] = spec
    return op


def _shrink_r1b_ref(in0, in1, s0, s1, imm2):
    v = in0.astype(np.float32) + imm2 * in1.astype(np.float32)
    return (np.maximum(v + s0, 0.0) + np.minimum(v + s1, 0.0)).astype(np.float32)


def _register_shrink_r1b():
    """Round-1 shrink with the momentum scale fused:
    out = relu(v+C0) + min(v+C1, 0),  v = Src0 + C2*Src1
    (in0 = E@s - rho'*x_old psum, in1 = x_cur, C2 = 1+rho',
    s0 = -wl, s1 = +wl)."""
    name = "SHRINK_R1B_FISTA_ANT"
    for op in dve_ops.OPS:
        if op.name == name:
            return op
    from concourse.dve_spec import C0, C1, C2
    v = Src0 + C2 * Src1
    spec = Spec(body=relu(v + C0) + minn(v + C1, Zero),
                reference=_shrink_r1b_ref)
    row = dve_ops._CUSTOM_DVE_ROW_BASE + len(dve_ops.OPS)
    assert row < 0x20
    dve_ops._SUB_OPCODE_FOR_NAME[name] = row
    shas = {}
    for ver in ("v3", "v4"):
        uops = lower(spec, ver=ver)
        shas[ver] = DveOpSpec(
            name=name, opcode=row, uops=uops, rd1_en=_has_src1(spec)
        ).sha(ver)
    op = dve_ops.DveOp(name, spec, subdim=False, uops_sha=shas)
    dve_ops.OPS.append(op)
    dve_ops.CUSTOM_DVE_SPECS[name] = spec
    return op


SHRINK = _register_shrink()
SHRINK_R1 = _register_shrink_r1()
SHRINK_R1B = _register_shrink_r1b()


# ---------------- host-side dictionary / constants ----------------
def _build_dic_np(rr, theta, T):
    """fp32 replica of reference.build_dictionary -> [T, 1+4N]."""
    rr = rr.astype(np.float32)
    theta = theta.astype(np.float32)
    i = np.arange(T, dtype=np.float32)[:, None]
    pow_p = np.power(rr[None, :], i, dtype=np.float32)
    sign = (1.0 - 2.0 * (np.arange(T) % 2)).astype(np.float32)[:, None]
    pow_m = sign * pow_p
    c = np.cos(i * theta[None, :], dtype=np.float32)
    s = np.sin(i * theta[None, :], dtype=np.float32)
    ones = np.ones((T, 1), np.float32)
    return np.concatenate([ones, pow_p * c, pow_m * c, pow_p * s, pow_m * s], axis=1)


def _rho_schedule(n):
    """rho_i = (t_i - 1)/t_{i+1}, t_0 = 1 (fp32 like the reference)."""
    rhos = []
    t = np.float32(1.0)
    for _ in range(n):
        t_new = np.float32((1.0 + math.sqrt(1.0 + 4.0 * float(t) * float(t))) / 2.0)
        rhos.append(np.float32((float(t) - 1.0) / float(t_new)))
        t = t_new
    return rhos


# pack16 layout (cols):
#   DT16   [128, 0:216]      D^T block j at cols [36j:36j+36]
#   ET16   [36, 216:984]     E^T = -D/L  (row t, col k)
#   I16    [128, 984:1112]   fp16 identity
#   I36    [36, 1112:1148]   fp16 identity (36)
#   negY16 [36, 1148:1212]   -Y local slice
#   negwl0 [128, 1212:1596]  -lam/L  (round-1 -wl)
#   w20    [128, 1596:1980]  2lam/L  (round-1 2wl)
C16 = 1980
# pack32 layout (cols):
#   I32     [128, 0:128]     fp32 identity
#   ones    [128, 128:129]   column of ones (partition reduce lhsT)
#   onesrow [0:1, 129:257]   row of ones (broadcast lhsT)
#   epsmask [128, 257:641]   EPS_W on valid K rows, 1e30 on padding
#   DT32    [128, 641:857]   D^T block j (fp32) at cols [641+36j : 641+36j+36]
C32 = 857


def _build_bass(rhos, lam_over_L):
    nc = bacc.Bacc("TRN2", target_bir_lowering=False, debug=False,
                   num_devices=N_CORES)
    pack32_d = nc.dram_tensor("pack32", [128, C32], f32, kind="ExternalInput")
    pack16_d = nc.dram_tensor("pack16", [128, C16], f16, kind="ExternalInput")
    xout_d = nc.dram_tensor("xout", [KP, PL], f32, kind="ExternalOutput")

    with tile.TileContext(nc) as tc:
        with (
            tc.tile_pool(name="const", bufs=1) as cpool,
            tc.tile_pool(name="xp", bufs=4) as xpool,
            tc.tile_pool(name="x16p", bufs=3) as x16pool,
            tc.tile_pool(name="y32p", bufs=2) as y32pool,
            tc.tile_pool(name="iscp", bufs=4) as iscpool,
            tc.tile_pool(name="xlop", bufs=3) as xlopool,
            tc.tile_pool(name="wtp", bufs=3) as wtpool,
            tc.tile_pool(name="s16p", bufs=3) as s16pool,
            tc.tile_pool(name="wlp", bufs=2) as wlpool,
            tc.tile_pool(name="rb", bufs=2) as rbpool,
            tc.tile_pool(name="psw", bufs=2, space="PSUM") as pswt,
            tc.tile_pool(name="psu", bufs=3, space="PSUM") as psu,
            tc.tile_pool(name="pss", bufs=1, space="PSUM") as pss,
            tc.tile_pool(name="dram", bufs=2, space="DRAM") as dram,
        ):
            pack32 = cpool.tile([128, C32], f32)
            pack16 = cpool.tile([128, C16], f16)
            nc.sync.dma_start(pack32[:], pack32_d[:])
            nc.sync.dma_start(pack16[:], pack16_d[:])

            I32 = pack32[:, 0:128]
            ones_col = pack32[:, 128:129]
            ones_row = pack32[0:1, 129:257]
            epsmask = pack32[:, 257:641]
            DT32 = pack32[:, 641:857]
            DT16 = pack16[:, 0:216]
            ET16 = pack16[0:T_DIM, 216:984]
            I16 = pack16[:, 984:1112]
            I36 = pack16[0:T_DIM, 1112:1148]
            negY16 = pack16[0:T_DIM, 1148:1212]
            negwl = pack16[:, 1212:1596]
            w2 = pack16[:, 1596:1980]

            # initial x state (read only via rho=0-scaled ops, but must be defined)
            x_cur = xpool.tile([128, SW], f32, tag="x")
            nc.vector.memset(x_cur[:], 0.0)
            zeros32 = cpool.tile([128, SW], f32)
            nc.vector.memset(zeros32[:], 0.0)
            x_prev = x_cur
            x16_cur = None
            x16_prev = None
            xlo_cur = None
            xlo_prev = None
            wt_sb_prev = None

            for rnd in range(N_REWEIGHT):
                r1 = rnd == 0
                isc_next = {}
                for it in range(MAX_ITER):
                    first = it == 0
                    rho_p = float(rhos[it - 1]) if it > 0 else 0.0  # rho_{i-1}

                    # --- generate next iteration's scaled fp16 identities now
                    #     so ScalarE finishes them well before the PE needs
                    #     them. cp = fp16(1+rho); cm = -(cp-1) exactly, so
                    #     cp + cm == 1 and the momentum fixed point is intact.
                    isc_cur = isc_next
                    isc_next = {}
                    if not r1 and it + 1 < MAX_ITER and float(rhos[it]) != 0.0:
                        cp32 = float(np.float32(np.float16(1.0 + rhos[it])))
                        iscp_n = iscpool.tile([128, 128], f16, tag="iscp")
                        nc.scalar.activation(iscp_n[:], I16, AF.Copy,
                                             scale=cp32)
                        iscm_n = iscpool.tile([128, 128], f16, tag="iscm")
                        nc.scalar.activation(iscm_n[:], I16, AF.Copy,
                                             scale=-(cp32 - 1.0))
                        isc_next["p"] = iscp_n
                        isc_next["m"] = iscm_n

                    # --- u psum prefix: scaled-identity / threshold MMs on
                    #     the compensated fp16 pair (xhi, xlo); emitted BEFORE
                    #     the w~ block so the in-order PE runs them inside the
                    #     shrink/cast window of the previous iteration ---
                    u = psu.tile([128, SW], f32, tag="u")
                    e_start = False
                    if r1:
                        # round 1: u = E@s16 only; y and thresholds fused in
                        # the shrink (y32 computed on DVE below)
                        e_start = True
                    if not r1:
                        if first:
                            nc.tensor.matmul(u[:], I16, negwl,
                                             start=True, stop=False)
                        elif rho_p == 0.0:
                            nc.tensor.matmul(u[:], I16, x16_cur[:],
                                             start=True, stop=False)
                            nc.tensor.matmul(u[:], I16, xlo_cur[:],
                                             start=False, stop=False)
                            nc.tensor.matmul(u[:], I16, negwl,
                                             start=False, stop=False)
                        else:
                            nc.tensor.matmul(u[:], isc_cur["m"][:], x16_prev[:],
                                             start=True, stop=False)
                            nc.tensor.matmul(u[:], isc_cur["m"][:], xlo_prev[:],
                                             start=False, stop=False)
                            nc.tensor.matmul(u[:], I16, negwl,
                                             start=False, stop=False)
                            nc.tensor.matmul(u[:], isc_cur["p"][:], x16_cur[:],
                                             start=False, stop=False)
                            nc.tensor.matmul(u[:], isc_cur["p"][:], xlo_cur[:],
                                             start=False, stop=False)

                    # --- wt = D@x16 - Y  [36, PL] psum ---
                    wt = pswt.tile([T_DIM, PL], f32, tag="wt")
                    if first:
                        nc.tensor.matmul(wt[:], I36, negY16, start=True, stop=True)
                    else:
                        for j in range(NB):
                            nc.tensor.matmul(
                                wt[:],
                                DT16[:, 36 * j:36 * j + 36],
                                x16_cur[:, PL * j:PL * j + PL],
                                start=(j == 0), stop=False,
                            )
                        nc.tensor.matmul(wt[:], I36, negY16, start=False, stop=True)

                    # --- s16 = (1+rho')wt - rho' wt_old (fp16) ---
                    s16 = s16pool.tile([T_DIM, PL], f16, tag="s16")
                    if first:
                        nc.scalar.activation(s16[:], wt[:], AF.Copy)
                    else:
                        nc.vector.ln_bwd_dx(
                            s16[:], wt[:], wt_sb_prev[:],
                            rho_p / (1.0 + rho_p), 0.0, 1.0 + rho_p,
                        )
                    if it < MAX_ITER - 1:
                        wt_sb = wtpool.tile([T_DIM, PL], f32, tag="wtsb")
                        nc.scalar.activation(wt_sb[:], wt[:], AF.Copy)
                    else:
                        wt_sb = None

                    # --- E-matmuls + fused shrink ---
                    for m in range(NB):
                        nc.tensor.matmul(
                            u[:, PL * m:PL * m + PL],
                            ET16[:, 128 * m:128 * m + 128], s16[:],
                            start=e_start, stop=True,
                        )
                    x_new = xpool.tile([128, SW], f32, tag="x")
                    if r1:
                        # y32 = (1+rho')x_cur - rho' x_prev on the DVE,
                        # consumed by the fused shrink's Src1
                        if first:
                            y32 = zeros32[:]
                        elif rho_p == 0.0:
                            y32 = x_cur[:]
                        else:
                            y32t = y32pool.tile([128, SW], f32, tag="y32")
                            nc.vector.ln_bwd_dx(
                                y32t[:], x_cur[:], x_prev[:],
                                rho_p / (1.0 + rho_p), 0.0, 1.0 + rho_p,
                            )
                            y32 = y32t[:]
                        # fp16 shrink first: it alone gates the next w~ chain
                        if it < MAX_ITER - 1:
                            x16_new = x16pool.tile([128, SW], f16, tag="x16")
                            nc.vector._custom_dve(
                                SHRINK_R1, out=x16_new[:], in0=u[:], in1=y32,
                                s0=float(-lam_over_L), s1=float(lam_over_L),
                            )
                        else:
                            x16_new = None
                        nc.vector._custom_dve(
                            SHRINK_R1, out=x_new[:], in0=u[:], in1=y32,
                            s0=float(-lam_over_L), s1=float(lam_over_L),
                        )
                    else:
                        nc.vector._custom_dve(SHRINK, out=x_new[:],
                                              in0=u[:], in1=w2)
                        # x16 cast feeds the next iteration's D-matmuls;
                        # xlo = fp16(x - x16) completes the compensated pair
                        if it < MAX_ITER - 1:
                            x16_new = x16pool.tile([128, SW], f16, tag="x16")
                            nc.vector.tensor_copy(x16_new[:], x_new[:])
                            xlo_new = xlopool.tile([128, SW], f16, tag="xlo")
                            nc.vector.tensor_sub(xlo_new[:], x_new[:],
                                                 x16_new[:])
                        else:
                            x16_new = None
                            xlo_new = None

                    if r1:
                        xlo_new = None
                    x_prev = x_cur
                    x_cur = x_new
                    x16_prev = x16_cur
                    x16_cur = x16_new
                    xlo_prev = xlo_cur
                    xlo_cur = xlo_new
                    wt_sb_prev = wt_sb

                # --- round boundary ---
                if rnd < N_REWEIGHT - 1:
                    # wr = 1/(|x| + epsmask)  (epsmask kills padded rows)
                    absx = rbpool.tile([128, SW], f32, tag="absx")
                    nc.scalar.activation(absx[:], x_cur[:], AF.Abs)
                    denom = rbpool.tile([128, SW], f32, tag="denom")
                    nc.vector.tensor_add(denom[:], absx[:], epsmask)
                    wr = rbpool.tile([128, SW], f32, tag="wr")
                    nc.vector.reciprocal_approx_fast(wr[:], denom[:])
                    # ssq per partition (Square activation with accumulate)
                    wr2 = rbpool.tile([128, SW], f32, tag="wr2")
                    pssum = rbpool.tile([128, 1], f32, tag="pssum")
                    nc.scalar.activation(wr2[:], wr[:], AF.Square,
                                         accum_out=pssum[:])
                    # partition reduce -> [1,1]
                    tot_ps = pss.tile([1, 1], f32, tag="tot")
                    nc.tensor.matmul(tot_ps[:], pssum[:], ones_col, start=True, stop=True)
                    tot_sb = rbpool.tile([1, 1], f32, tag="totsb")
                    nc.vector.tensor_copy(tot_sb[:], tot_ps[:])
                    # AllReduce
                    cc_in = dram.tile([1, 1], f32, tag="ccin")
                    cc_out = dram.tile([1, 1], f32, tag="ccout")
                    nc.sync.dma_start(cc_in[:], tot_sb[:])
                    if os.environ.get("SKIP_CC", "0") == "1":
                        nc.sync.dma_start(cc_out[:], cc_in[:])
                    else:
                        nc.gpsimd.collective_compute(
                            "AllReduce", mybir.AluOpType.add,
                            replica_groups=[list(range(N_CORES))],
                            ins=[cc_in.opt()], outs=[cc_out.opt()],
                        )
                    tot_g = rbpool.tile([1, 1], f32, tag="totg")
                    nc.sync.dma_start(tot_g[:], cc_out[:])
                    # rnorm = sqrt(1/tot)
                    inv_t = rbpool.tile([1, 1], f32, tag="invt")
                    nc.vector.reciprocal(inv_t[:], tot_g[:])
                    rnorm = rbpool.tile([1, 1], f32, tag="rnorm")
                    nc.scalar.activation(rnorm[:], inv_t[:], AF.Sqrt)
                    # broadcast to [128,1]
                    rn_ps = pss.tile([128, 1], f32, tag="rnps")
                    nc.tensor.matmul(rn_ps[:], ones_row, rnorm[:], start=True, stop=True)
                    rn128 = rbpool.tile([128, 1], f32, tag="rn128")
                    nc.vector.tensor_copy(rn128[:], rn_ps[:])
                    # new thresholds (fp16): negwl = -wr*rn*lam/L ; w2 = 2*wr*rn*lam/L
                    negwl_n = wlpool.tile([128, SW], f16, tag="negwl")
                    w2_n = wlpool.tile([128, SW], f16, tag="w2")
                    nc.vector.tensor_scalar(
                        out=negwl_n[:], in0=wr[:], scalar1=rn128[:],
                        scalar2=float(-lam_over_L), op0=mybir.AluOpType.mult,
                        op1=mybir.AluOpType.mult,
                    )
                    nc.vector.tensor_scalar(
                        out=w2_n[:], in0=wr[:], scalar1=rn128[:],
                        scalar2=float(2.0 * lam_over_L), op0=mybir.AluOpType.mult,
                        op1=mybir.AluOpType.mult,
                    )
                    negwl = negwl_n[:]
                    w2 = w2_n[:]
                    x16_cur = None
                    x16_prev = None
                    xlo_cur = None
                    xlo_prev = None
                    wt_sb_prev = None

            # --- write out x [128, SW] -> [KP, PL] (one DMA per K block) ---
            for j in range(NB):
                nc.sync.dma_start(
                    xout_d[128 * j:128 * j + 128, :],
                    x_cur[:, PL * j:PL * j + PL],
                )

    nc.compile()
    return nc


_CACHE = {}


def _get_nc(rhos, lam_over_L):
    key = round(float(lam_over_L), 10)
    if key not in _CACHE:
        _CACHE[key] = _build_bass(rhos, lam_over_L)
    return _CACHE[key]


def _prepare(x, rr, theta, T):
    x = np.asarray(x, dtype=np.float32)
    rr = np.asarray(rr, dtype=np.float32)
    theta = np.asarray(theta, dtype=np.float32)
    T = int(np.asarray(T))
    assert T == T_DIM and x.shape == (1, T_DIM, P_DIM)

    D = _build_dic_np(rr, theta, T)            # [36, 641]
    assert D.shape == (T_DIM, K_DIM)
    DDt = (D.astype(np.float64) @ D.astype(np.float64).T)
    L = float(np.linalg.eigvalsh(DDt)[-1])
    lam_over_L = np.float32(LAM / L)

    Dp = np.zeros((T_DIM, KP), np.float32)
    Dp[:, :K_DIM] = D
    ET = (-Dp / np.float32(L)).astype(np.float32)   # E^T = -D/L [36, KP]
    DTp = Dp.T.astype(np.float16)                   # [KP, 36]

    rhos = _rho_schedule(MAX_ITER)

    # pack32 (shared across cores)
    pack32 = np.zeros((128, C32), np.float32)
    pack32[:, 0:128] = np.eye(128, dtype=np.float32)
    pack32[:, 128] = 1.0
    pack32[0, 129:257] = 1.0
    em = np.full((KP,), 1e30, np.float32)
    em[:K_DIM] = EPS_W
    # epsmask layout must match state [128, SW]: block j at cols [PL*j : PL*j+PL]
    emask = np.empty((128, SW), np.float32)
    for j in range(NB):
        emask[:, PL * j:PL * j + PL] = em[128 * j:128 * j + 128][:, None]
    pack32[:, 257:641] = emask
    DTp32 = Dp.T.astype(np.float32)
    for j in range(NB):
        pack32[:, 641 + 36 * j:641 + 36 * j + 36] = DTp32[128 * j:128 * j + 128, :]

    # pack16 core-independent part
    base16 = np.zeros((128, C16), np.float16)
    for j in range(NB):
        base16[:, 36 * j:36 * j + 36] = DTp[128 * j:128 * j + 128, :]
    base16[0:T_DIM, 216:984] = ET.astype(np.float16)
    base16[:, 984:1112] = np.eye(128, dtype=np.float16)
    base16[0:T_DIM, 1112:1148] = np.eye(T_DIM, dtype=np.float16)
    wl0 = np.float32(lam_over_L)
    base16[:, 1212:1596] = -wl0
    base16[:, 1596:1980] = 2.0 * wl0

    in_maps = []
    for c in range(N_CORES):
        p16 = base16.copy()
        Yl = x[0][:, PL * c:PL * c + PL]
        p16[0:T_DIM, 1148:1212] = (-Yl).astype(np.float16)
        in_maps.append({"pack32": pack32, "pack16": p16})

    return D, rhos, lam_over_L, in_maps


def _run(x, rr, theta, T, trace=False):
    D, rhos, lam_over_L, in_maps = _prepare(x, rr, theta, T)
    nc = _get_nc(rhos, lam_over_L)
    res = run_bass_kernel_spmd(
        nc, in_maps, core_ids=list(range(N_CORES)), trace=trace
    )
    final = np.empty((1, K_DIM, P_DIM), np.float32)
    for c in range(N_CORES):
        xo = res.results[c]["xout"]            # [KP, PL]
        final[0][:, PL * c:PL * c + PL] = xo[:K_DIM]
    return (final, D), res


def kernel(x, rr, theta, T):
    out, _ = _run(x, rr, theta, T, trace=False)
    return out


# revision 47
# speedup vs baseline: 1.0392x; 1.0392x over previous
"""DyanEncoder (FISTA sparse coding) Trainium2 kernel.

Math (3 reweighting rounds x 200 FISTA iterations, all sequential):
    D [36,641],  L = ||D^T D||_2,  E = -D^T/L
    u_i = y_i + E (D y_i - Y) - wl          (rank-36 cascade, c folded in)
    x_{i+1} = relu(u_i) + min(u_i + 2wl, 0) (soft-threshold)
    y_{i+1} = (1+rho_i) x_{i+1} - rho_i x_i (momentum; rho precomputed)

Sharding: data-parallel over P=512 columns, 64 per core, zero comm in the
iteration; one scalar AllReduce per w-renormalization (2 total).

Per-iteration dataflow (per core; K padded 641->768, state [128, 6*64];
momentum recombined in the tiny [36,64] space via wt_k = D@x_k - Y, using
(1+r)wt_new - r wt_old = D@y - Y since (1+r)-r = 1):
    wt = D@x16 - Y                    (6 fp16 MMs + I36@(-Y) -> PSUM [36,64])
    s16 = (1+rho')wt - rho' wt_old    (ln_bwd_dx custom DVE op, fp16 out)
    u(round 1)  = E@s16               (6 fp16 MMs; thresholds are scalar)
    u(rounds2+) = cp*I@x_pair + cm*I@x_old_pair - wl + E@s16
                  (scaled fp16 identities, ScalarE-generated one iteration
                   ahead; x as compensated fp16 hi+lo pair; cp = fp16(1+rho),
                   cm = -(cp-1) exactly so cp+cm == 1)
    x = SHRINK custom DVE op: round 1 relu(u+y+C0)+min(u+y+C1,0) with y via
        Src1 (emitted twice: fp16 out gates the next w~ chain, fp32 after);
        rounds 2+ relu(u)+min(u+w2,0) with tensor w2, then x16/xlo casts.
Numerical ground rules (measured): the y/identity path must keep x in
fp32-equivalent form (plain fp16 there diverges 0.25 rel err via momentum
noise amplification); the residual cascade tolerates fp16 (1.0e-3 final).
Round boundary: wr = 1/(|x|+eps) (fast reciprocal), ssq via Square-ACT
accum + ones-matmul partition reduce, AllReduce, rsqrt, new wl/w2 in fp16.
"""

import math
import os

import numpy as np

import concourse.bass as bass
import concourse.mybir as mybir
import concourse.tile as tile
from concourse import bacc
from concourse.bass_utils import run_bass_kernel_spmd

import concourse.dve_ops as dve_ops
from concourse.dve_spec import Spec, Src0, Src1, Zero, relu, minn, lower, _has_src1
from concourse.dve_uop import DveOpSpec

f32, f16 = mybir.dt.float32, mybir.dt.float16
AF = mybir.ActivationFunctionType

N_CORES = 8
MAX_ITER = int(os.environ.get("FISTA_ITERS", "200"))
N_REWEIGHT = int(os.environ.get("FISTA_ROUNDS", "3"))
EPS_W = 0.01
LAM = 0.1
T_DIM = 36
K_DIM = 641
P_DIM = 512
KP = 768            # padded K (6 blocks of 128)
NB = 6              # K blocks
PL = P_DIM // N_CORES   # 64 local columns
SW = NB * PL        # 384 state width ([128, SW] layout, block j at cols j*PL)


# ---------------- custom fused shrink op ----------------
def _shrink_ref(in0, in1, s0, s1, imm2):
    u = in0.astype(np.float32)
    w2 = in1.astype(np.float32)
    return (np.maximum(u, 0.0) + np.minimum(u + w2, 0.0)).astype(np.float32)


def _register_shrink():
    name = "SHRINK_FISTA_ANT"
    for op in dve_ops.OPS:
        if op.name == name:
            return op
    spec = Spec(body=relu(Src0) + minn(Src0 + Src1, Zero), reference=_shrink_ref)
    row = dve_ops._CUSTOM_DVE_ROW_BASE + len(dve_ops.OPS)
    assert row < 0x20
    dve_ops._SUB_OPCODE_FOR_NAME[name] = row
    shas = {}
    for ver in ("v3", "v4"):
        uops = lower(spec, ver=ver)
        shas[ver] = DveOpSpec(
            name=name, opcode=row, uops=uops, rd1_en=_has_src1(spec)
        ).sha(ver)
    op = dve_ops.DveOp(name, spec, subdim=False, uops_sha=shas)
    dve_ops.OPS.append(op)
    dve_ops.CUSTOM_DVE_SPECS[name] = spec
    return op


def _shrink_r1_ref(in0, in1, s0, s1, imm2):
    u = in0.astype(np.float32) + in1.astype(np.float32)
    return (np.maximum(u + s0, 0.0) + np.minimum(u + s1, 0.0)).astype(np.float32)


def _register_shrink_r1():
    """Round-1 shrink with SCALAR thresholds and the +y folded in:
    out = relu(Src0+Src1+C0) + min(Src0+Src1+C1, 0)
    (in0 = E@s psum, in1 = y32, s0 = -wl, s1 = +wl)."""
    name = "SHRINK_R1_FISTA_ANT"
    for op in dve_ops.OPS:
        if op.name == name:
            return op
    u = Src0 + Src1
    from concourse.dve_spec import C0, C1
    spec = Spec(body=relu(u + C0) + minn(u + C1, Zero), reference=_shrink_r1_ref)
    row = dve_ops._CUSTOM_DVE_ROW_BASE + len(dve_ops.OPS)
    assert row < 0x20
    dve_ops._SUB_OPCODE_FOR_NAME[name] = row
    shas = {}
    for ver in ("v3", "v4"):
        uops = lower(spec, ver=ver)
        shas[ver] = DveOpSpec(
            name=name, opcode=row, uops=uops, rd1_en=_has_src1(spec)
        ).sha(ver)
    op = dve_ops.DveOp(name, spec, subdim=False, uops_sha=shas)
    dve_ops.OPS.append(op)
    dve_ops.CUSTOM_DVE_SPECS[name] = spec
    return op


def _shrink_r1b_ref(in0, in1, s0, s1, imm2):
    v = in0.astype(np.float32) + imm2 * in1.astype(np.float32)
    return (np.maximum(v + s0, 0.0) + np.minimum(v + s1, 0.0)).astype(np.float32)


def _register_shrink_r1b():
    """Round-1 shrink with the momentum scale fused:
    out = relu(v+C0) + min(v+C1, 0),  v = Src0 + C2*Src1
    (in0 = E@s - rho'*x_old psum, in1 = x_cur, C2 = 1+rho',
    s0 = -wl, s1 = +wl)."""
    name = "SHRINK_R1B_FISTA_ANT"
    for op in dve_ops.OPS:
        if op.name == name:
            return op
    from concourse.dve_spec import C0, C1, C2
    v = Src0 + C2 * Src1
    spec = Spec(body=relu(v + C0) + minn(v + C1, Zero),
                reference=_shrink_r1b_ref)
    row = dve_ops._CUSTOM_DVE_ROW_BASE + len(dve_ops.OPS)
    assert row < 0x20
    dve_ops._SUB_OPCODE_FOR_NAME[name] = row
    shas = {}
    for ver in ("v3", "v4"):
        uops = lower(spec, ver=ver)
        shas[ver] = DveOpSpec(
            name=name, opcode=row, uops=uops, rd1_en=_has_src1(spec)
        ).sha(ver)
    op = dve_ops.DveOp(name, spec, subdim=False, uops_sha=shas)
    dve_ops.OPS.append(op)
    dve_ops.CUSTOM_DVE_SPECS[name] = spec
    return op


SHRINK = _register_shrink()
SHRINK_R1 = _register_shrink_r1()
SHRINK_R1B = _register_shrink_r1b()


# ---------------- host-side dictionary / constants ----------------
def _build_dic_np(rr, theta, T):
    """fp32 replica of reference.build_dictionary -> [T, 1+4N]."""
    rr = rr.astype(np.float32)
    theta = theta.astype(np.float32)
    i = np.arange(T, dtype=np.float32)[:, None]
    pow_p = np.power(rr[None, :], i, dtype=np.float32)
    sign = (1.0 - 2.0 * (np.arange(T) % 2)).astype(np.float32)[:, None]
    pow_m = sign * pow_p
    c = np.cos(i * theta[None, :], dtype=np.float32)
    s = np.sin(i * theta[None, :], dtype=np.float32)
    ones = np.ones((T, 1), np.float32)
    return np.concatenate([ones, pow_p * c, pow_m * c, pow_p * s, pow_m * s], axis=1)


def _rho_schedule(n):
    """rho_i = (t_i - 1)/t_{i+1}, t_0 = 1 (fp32 like the reference)."""
    rhos = []
    t = np.float32(1.0)
    for _ in range(n):
        t_new = np.float32((1.0 + math.sqrt(1.0 + 4.0 * float(t) * float(t))) / 2.0)
        rhos.append(np.float32((float(t) - 1.0) / float(t_new)))
        t = t_new
    return rhos


# pack16 layout (cols):
#   DT16   [128, 0:216]      D^T block j at cols [36j:36j+36]
#   ET16   [36, 216:984]     E^T = -D/L  (row t, col k)
#   I16    [128, 984:1112]   fp16 identity
#   I36    [36, 1112:1148]   fp16 identity (36)
#   negY16 [36, 1148:1212]   -Y local slice
#   negwl0 [128, 1212:1596]  -lam/L  (round-1 -wl)
#   w20    [128, 1596:1980]  2lam/L  (round-1 2wl)
C16 = 1980
# pack32 layout (cols):
#   I32     [128, 0:128]     fp32 identity
#   ones    [128, 128:129]   column of ones (partition reduce lhsT)
#   onesrow [0:1, 129:257]   row of ones (broadcast lhsT)
#   epsmask [128, 257:641]   EPS_W on valid K rows, 1e30 on padding
#   DT32    [128, 641:857]   D^T block j (fp32) at cols [641+36j : 641+36j+36]
C32 = 857


def _build_bass(rhos, lam_over_L):
    nc = bacc.Bacc("TRN2", target_bir_lowering=False, debug=False,
                   num_devices=N_CORES)
    pack32_d = nc.dram_tensor("pack32", [128, C32], f32, kind="ExternalInput")
    pack16_d = nc.dram_tensor("pack16", [128, C16], f16, kind="ExternalInput")
    xout_d = nc.dram_tensor("xout", [KP, PL], f32, kind="ExternalOutput")

    with tile.TileContext(nc) as tc:
        with (
            tc.tile_pool(name="const", bufs=1) as cpool,
            tc.tile_pool(name="xp", bufs=4) as xpool,
            tc.tile_pool(name="x16p", bufs=3) as x16pool,
            tc.tile_pool(name="y32p", bufs=2) as y32pool,
            tc.tile_pool(name="iscp", bufs=4) as iscpool,
            tc.tile_pool(name="xlop", bufs=3) as xlopool,
            tc.tile_pool(name="wtp", bufs=3) as wtpool,
            tc.tile_pool(name="s16p", bufs=3) as s16pool,
            tc.tile_pool(name="wlp", bufs=2) as wlpool,
            tc.tile_pool(name="rb", bufs=2) as rbpool,
            tc.tile_pool(name="psw", bufs=2, space="PSUM") as pswt,
            tc.tile_pool(name="psu", bufs=3, space="PSUM") as psu,
            tc.tile_pool(name="pss", bufs=1, space="PSUM") as pss,
            tc.tile_pool(name="dram", bufs=2, space="DRAM") as dram,
        ):
            pack32 = cpool.tile([128, C32], f32)
            pack16 = cpool.tile([128, C16], f16)
            nc.sync.dma_start(pack32[:], pack32_d[:])
            nc.sync.dma_start(pack16[:], pack16_d[:])

            I32 = pack32[:, 0:128]
            ones_col = pack32[:, 128:129]
            ones_row = pack32[0:1, 129:257]
            epsmask = pack32[:, 257:641]
            DT32 = pack32[:, 641:857]
            DT16 = pack16[:, 0:216]
            ET16 = pack16[0:T_DIM, 216:984]
            I16 = pack16[:, 984:1112]
            I36 = pack16[0:T_DIM, 1112:1148]
            negY16 = pack16[0:T_DIM, 1148:1212]
            negwl = pack16[:, 1212:1596]
            w2 = pack16[:, 1596:1980]

            # initial x state (read only via rho=0-scaled ops, but must be defined)
            x_cur = xpool.tile([128, SW], f32, tag="x")
            nc.vector.memset(x_cur[:], 0.0)
            zeros32 = cpool.tile([128, SW], f32)
            nc.vector.memset(zeros32[:], 0.0)
            x_prev = x_cur
            x16_cur = None
            x16_prev = None
            xlo_cur = None
            xlo_prev = None
            wt_sb_prev = None

            for rnd in range(N_REWEIGHT):
                r1 = rnd == 0
                isc_next = {}
                for it in range(MAX_ITER):
                    first = it == 0
                    rho_p = float(rhos[it - 1]) if it > 0 else 0.0  # rho_{i-1}

                    # --- generate next iteration's scaled fp16 identities now
                    #     so ScalarE finishes them well before the PE needs
                    #     them. cp = fp16(1+rho); cm = -(cp-1) exactly, so
                    #     cp + cm == 1 and the momentum fixed point is intact.
                    isc_cur = isc_next
                    isc_next = {}
                    if not r1 and it + 1 < MAX_ITER and float(rhos[it]) != 0.0:
                        cp32 = float(np.float32(np.float16(1.0 + rhos[it])))
                        iscp_n = iscpool.tile([128, 128], f16, tag="iscp")
                        nc.scalar.activation(iscp_n[:], I16, AF.Copy,
                                             scale=cp32)
                        iscm_n = iscpool.tile([128, 128], f16, tag="iscm")
                        nc.scalar.activation(iscm_n[:], I16, AF.Copy,
                                             scale=-(cp32 - 1.0))
                        isc_next["p"] = iscp_n
                        isc_next["m"] = iscm_n

                    # --- u psum prefix: scaled-identity / threshold MMs on
                    #     the compensated fp16 pair (xhi, xlo); emitted BEFORE
                    #     the w~ block so the in-order PE runs them inside the
                    #     shrink/cast window of the previous iteration ---
                    u = psu.tile([128, SW], f32, tag="u")
                    e_start = False
                    if r1:
                        # round 1: u = E@s16 only; y and thresholds fused in
                        # the shrink (y32 computed on DVE below)
                        e_start = True
                    # --- wt = D@x16 - Y  [36, PL] psum ---
                    wt = pswt.tile([T_DIM, PL], f32, tag="wt")
                    if first:
                        nc.tensor.matmul(wt[:], I36, negY16, start=True, stop=True)
                    else:
                        for j in range(NB):
                            nc.tensor.matmul(
                                wt[:],
                                DT16[:, 36 * j:36 * j + 36],
                                x16_cur[:, PL * j:PL * j + PL],
                                start=(j == 0), stop=False,
                            )
                        nc.tensor.matmul(wt[:], I36, negY16, start=False, stop=True)

                    # --- s16 = (1+rho')wt - rho' wt_old (fp16) ---
                    s16 = s16pool.tile([T_DIM, PL], f16, tag="s16")
                    if first:
                        nc.scalar.activation(s16[:], wt[:], AF.Copy)
                    else:
                        nc.vector.ln_bwd_dx(
                            s16[:], wt[:], wt_sb_prev[:],
                            rho_p / (1.0 + rho_p), 0.0, 1.0 + rho_p,
                        )
                    if it < MAX_ITER - 1:
                        wt_sb = wtpool.tile([T_DIM, PL], f32, tag="wtsb")
                        nc.scalar.activation(wt_sb[:], wt[:], AF.Copy)
                    else:
                        wt_sb = None

                    if not r1:
                        if first:
                            nc.tensor.matmul(u[:], I16, negwl,
                                             start=True, stop=False)
                        elif rho_p == 0.0:
                            nc.tensor.matmul(u[:], I16, x16_cur[:],
                                             start=True, stop=False)
                            nc.tensor.matmul(u[:], I16, xlo_cur[:],
                                             start=False, stop=False)
                            nc.tensor.matmul(u[:], I16, negwl,
                                             start=False, stop=False)
                        else:
                            nc.tensor.matmul(u[:], isc_cur["m"][:], x16_prev[:],
                                             start=True, stop=False)
                            nc.tensor.matmul(u[:], isc_cur["m"][:], xlo_prev[:],
                                             start=False, stop=False)
                            nc.tensor.matmul(u[:], I16, negwl,
                                             start=False, stop=False)
                            nc.tensor.matmul(u[:], isc_cur["p"][:], x16_cur[:],
                                             start=False, stop=False)
                            nc.tensor.matmul(u[:], isc_cur["p"][:], xlo_cur[:],
                                             start=False, stop=False)

                    # --- E-matmuls + fused shrink ---
                    for m in range(NB):
                        nc.tensor.matmul(
                            u[:, PL * m:PL * m + PL],
                            ET16[:, 128 * m:128 * m + 128], s16[:],
                            start=e_start, stop=True,
                        )
                    x_new = xpool.tile([128, SW], f32, tag="x")
                    if r1:
                        # y32 = (1+rho')x_cur - rho' x_prev on the DVE,
                        # consumed by the fused shrink's Src1
                        if first:
                            y32 = zeros32[:]
                        elif rho_p == 0.0:
                            y32 = x_cur[:]
                        else:
                            y32t = y32pool.tile([128, SW], f32, tag="y32")
                            nc.vector.ln_bwd_dx(
                                y32t[:], x_cur[:], x_prev[:],
                                rho_p / (1.0 + rho_p), 0.0, 1.0 + rho_p,
                            )
                            y32 = y32t[:]
                        # fp16 shrink first: it alone gates the next w~ chain
                        if it < MAX_ITER - 1:
                            x16_new = x16pool.tile([128, SW], f16, tag="x16")
                            nc.vector._custom_dve(
                                SHRINK_R1, out=x16_new[:], in0=u[:], in1=y32,
                                s0=float(-lam_over_L), s1=float(lam_over_L),
                            )
                        else:
                            x16_new = None
                        nc.vector._custom_dve(
                            SHRINK_R1, out=x_new[:], in0=u[:], in1=y32,
                            s0=float(-lam_over_L), s1=float(lam_over_L),
                        )
                    else:
                        nc.vector._custom_dve(SHRINK, out=x_new[:],
                                              in0=u[:], in1=w2)
                        # x16 cast feeds the next iteration's D-matmuls;
                        # xlo = fp16(x - x16) completes the compensated pair
                        if it < MAX_ITER - 1:
                            x16_new = x16pool.tile([128, SW], f16, tag="x16")
                            nc.vector.tensor_copy(x16_new[:], x_new[:])
                            xlo_new = xlopool.tile([128, SW], f16, tag="xlo")
                            nc.vector.tensor_sub(xlo_new[:], x_new[:],
                                                 x16_new[:])
                        else:
                            x16_new = None
                            xlo_new = None

                    if r1:
                        xlo_new = None
                    x_prev = x_cur
                    x_cur = x_new
                    x16_prev = x16_cur
                    x16_cur = x16_new
                    xlo_prev = xlo_cur
                    xlo_cur = xlo_new
                    wt_sb_prev = wt_sb

                # --- round boundary ---
                if rnd < N_REWEIGHT - 1:
                    # wr = 1/(|x| + epsmask)  (epsmask kills padded rows)
                    absx = rbpool.tile([128, SW], f32, tag="absx")
                    nc.scalar.activation(absx[:], x_cur[:], AF.Abs)
                    denom = rbpool.tile([128, SW], f32, tag="denom")
                    nc.vector.tensor_add(denom[:], absx[:], epsmask)
                    wr = rbpool.tile([128, SW], f32, tag="wr")
                    nc.vector.reciprocal_approx_fast(wr[:], denom[:])
                    # ssq per partition (Square activation with accumulate)
                    wr2 = rbpool.tile([128, SW], f32, tag="wr2")
                    pssum = rbpool.tile([128, 1], f32, tag="pssum")
                    nc.scalar.activation(wr2[:], wr[:], AF.Square,
                                         accum_out=pssum[:])
                    # partition reduce -> [1,1]
                    tot_ps = pss.tile([1, 1], f32, tag="tot")
                    nc.tensor.matmul(tot_ps[:], pssum[:], ones_col, start=True, stop=True)
                    tot_sb = rbpool.tile([1, 1], f32, tag="totsb")
                    nc.vector.tensor_copy(tot_sb[:], tot_ps[:])
                    # AllReduce
                    cc_in = dram.tile([1, 1], f32, tag="ccin")
                    cc_out = dram.tile([1, 1], f32, tag="ccout")
                    nc.sync.dma_start(cc_in[:], tot_sb[:])
                    if os.environ.get("SKIP_CC", "0") == "1":
                        nc.sync.dma_start(cc_out[:], cc_in[:])
                    else:
                        nc.gpsimd.collective_compute(
                            "AllReduce", mybir.AluOpType.add,
                            replica_groups=[list(range(N_CORES))],
                            ins=[cc_in.opt()], outs=[cc_out.opt()],
                        )
                    tot_g = rbpool.tile([1, 1], f32, tag="totg")
                    nc.sync.dma_start(tot_g[:], cc_out[:])
                    # rnorm = sqrt(1/tot)
                    inv_t = rbpool.tile([1, 1], f32, tag="invt")
                    nc.vector.reciprocal(inv_t[:], tot_g[:])
                    rnorm = rbpool.tile([1, 1], f32, tag="rnorm")
                    nc.scalar.activation(rnorm[:], inv_t[:], AF.Sqrt)
                    # broadcast to [128,1]
                    rn_ps = pss.tile([128, 1], f32, tag="rnps")
                    nc.tensor.matmul(rn_ps[:], ones_row, rnorm[:], start=True, stop=True)
                    rn128 = rbpool.tile([128, 1], f32, tag="rn128")
                    nc.vector.tensor_copy(rn128[:], rn_ps[:])
                    # new thresholds (fp16): negwl = -wr*rn*lam/L ; w2 = 2*wr*rn*lam/L
                    negwl_n = wlpool.tile([128, SW], f16, tag="negwl")
                    w2_n = wlpool.tile([128, SW], f16, tag="w2")
                    nc.vector.tensor_scalar(
                        out=negwl_n[:], in0=wr[:], scalar1=rn128[:],
                        scalar2=float(-lam_over_L), op0=mybir.AluOpType.mult,
                        op1=mybir.AluOpType.mult,
                    )
                    nc.vector.tensor_scalar(
                        out=w2_n[:], in0=wr[:], scalar1=rn128[:],
                        scalar2=float(2.0 * lam_over_L), op0=mybir.AluOpType.mult,
                        op1=mybir.AluOpType.mult,
                    )
                    negwl = negwl_n[:]
                    w2 = w2_n[:]
                    x16_cur = None
                    x16_prev = None
                    xlo_cur = None
                    xlo_prev = None
                    wt_sb_prev = None

            # --- write out x [128, SW] -> [KP, PL] (one DMA per K block) ---
            for j in range(NB):
                nc.sync.dma_start(
                    xout_d[128 * j:128 * j + 128, :],
                    x_cur[:, PL * j:PL * j + PL],
                )

    nc.compile()
    return nc


_CACHE = {}


def _get_nc(rhos, lam_over_L):
    key = round(float(lam_over_L), 10)
    if key not in _CACHE:
        _CACHE[key] = _build_bass(rhos, lam_over_L)
    return _CACHE[key]


def _prepare(x, rr, theta, T):
    x = np.asarray(x, dtype=np.float32)
    rr = np.asarray(rr, dtype=np.float32)
    theta = np.asarray(theta, dtype=np.float32)
    T = int(np.asarray(T))
    assert T == T_DIM and x.shape == (1, T_DIM, P_DIM)

    D = _build_dic_np(rr, theta, T)            # [36, 641]
    assert D.shape == (T_DIM, K_DIM)
    DDt = (D.astype(np.float64) @ D.astype(np.float64).T)
    L = float(np.linalg.eigvalsh(DDt)[-1])
    lam_over_L = np.float32(LAM / L)

    Dp = np.zeros((T_DIM, KP), np.float32)
    Dp[:, :K_DIM] = D
    ET = (-Dp / np.float32(L)).astype(np.float32)   # E^T = -D/L [36, KP]
    DTp = Dp.T.astype(np.float16)                   # [KP, 36]

    rhos = _rho_schedule(MAX_ITER)

    # pack32 (shared across cores)
    pack32 = np.zeros((128, C32), np.float32)
    pack32[:, 0:128] = np.eye(128, dtype=np.float32)
    pack32[:, 128] = 1.0
    pack32[0, 129:257] = 1.0
    em = np.full((KP,), 1e30, np.float32)
    em[:K_DIM] = EPS_W
    # epsmask layout must match state [128, SW]: block j at cols [PL*j : PL*j+PL]
    emask = np.empty((128, SW), np.float32)
    for j in range(NB):
        emask[:, PL * j:PL * j + PL] = em[128 * j:128 * j + 128][:, None]
    pack32[:, 257:641] = emask
    DTp32 = Dp.T.astype(np.float32)
    for j in range(NB):
        pack32[:, 641 + 36 * j:641 + 36 * j + 36] = DTp32[128 * j:128 * j + 128, :]

    # pack16 core-independent part
    base16 = np.zeros((128, C16), np.float16)
    for j in range(NB):
        base16[:, 36 * j:36 * j + 36] = DTp[128 * j:128 * j + 128, :]
    base16[0:T_DIM, 216:984] = ET.astype(np.float16)
    base16[:, 984:1112] = np.eye(128, dtype=np.float16)
    base16[0:T_DIM, 1112:1148] = np.eye(T_DIM, dtype=np.float16)
    wl0 = np.float32(lam_over_L)
    base16[:, 1212:1596] = -wl0
    base16[:, 1596:1980] = 2.0 * wl0

    in_maps = []
    for c in range(N_CORES):
        p16 = base16.copy()
        Yl = x[0][:, PL * c:PL * c + PL]
        p16[0:T_DIM, 1148:1212] = (-Yl).astype(np.float16)
        in_maps.append({"pack32": pack32, "pack16": p16})

    return D, rhos, lam_over_L, in_maps


def _run(x, rr, theta, T, trace=False):
    D, rhos, lam_over_L, in_maps = _prepare(x, rr, theta, T)
    nc = _get_nc(rhos, lam_over_L)
    res = run_bass_kernel_spmd(
        nc, in_maps, core_ids=list(range(N_CORES)), trace=trace
    )
    final = np.empty((1, K_DIM, P_DIM), np.float32)
    for c in range(N_CORES):
        xo = res.results[c]["xout"]            # [KP, PL]
        final[0][:, PL * c:PL * c + PL] = xo[:K_DIM]
    return (final, D), res


def kernel(x, rr, theta, T):
    out, _ = _run(x, rr, theta, T, trace=False)
    return out
